# revision 9
# baseline (speedup 1.0000x reference)
"""Trainium2 Bass kernel for nn_BaseEmbedder (retrieval_knn).

For each of 4096 query embeddings: find the 5 nearest of 65536 db embeddings
(Euclidean) and produce the inverse-distance-weighted sum of their auxiliary
features.  SPMD on 8 NeuronCores: queries sharded 512/core, db+aux replicated.

v2 design (vs the single-row-group v1):
  - Scan (bf16): negS[q,j] = q.x_j - 0.5|x_j|^2 via K=34 augmented bf16
    matmuls.  The PE runs TWO matmuls concurrently via 2-way row-group
    packing: weights live at array rows 0-33 (tile_position (0,0)) and rows
    64-97 (tile_position (64,0)).  db columns are split host-side into two
    DRAM slabs (dbA = first 1024 of each 2048-row tile, dbB = second 1024)
    and streamed into one [128, 1024] SBUF tile per scan tile (partitions
    0:34 and 64:98).
  - PSUM drain runs on three parallel paths, chosen statically per 4096-col
    "super unit" (su = 2 psum tiles):
      P2: DVE folds directly from PSUM (TT max of tile halves), Pool (gpsimd)
          finishes the bf16 fold chain.
      P3: ACT evacuates both tiles to bf16, DVE runs the whole fold chain.
      P4: ACT evacuates, DVE does the first bf16 fold, Pool finishes.
    All paths produce the same slot map: z8 slot (su, u) covers db rows
    su*4096 + u + 512*m, m=0..7 (every fold level halves the linear psum
    order), so the host pair table is path-independent.
  - Selection: MAX8 + FIND_INDEX8 over z8 [128, 8192] f32 per q-tile, as in
    v1 (custom ops are f32-only on HW).
  - Exact refinement (f32): per winning slot a host-prepared row
    [8x (x,aux), 8x |x|^2] is gathered via per-partition indirect DMA; exact
    distances for all 64 candidates are recomputed; top-5 by threshold;
    weights 1/(d+eps) normalized; weighted aux sum.  The aux-side product and
    reduction run on Pool (gpsimd) to unload DVE.

The bf16 scan only nominates candidates; all selection/weight math is exact
f32, so the result matches the f32 reference to ~1e-6.
"""

import numpy as np
import ml_dtypes

from concourse import bass, mybir
from concourse.tile import TileContext
from concourse.bass_utils import run_bass_kernel_spmd

F32 = mybir.dt.float32
BF16 = mybir.dt.bfloat16
U32 = mybir.dt.uint32
I32 = mybir.dt.int32

N_CORES = 8
NQ = 4096
NDB = 65536
D = 32
DAUG = 34   # 32 dims + bias row + bias-residual row (bf16 split)
K = 5
EPS = 1e-6

NQ_CORE = NQ // N_CORES          # 512
N_QT = NQ_CORE // 128            # 4 q-tiles
TILE = 2048                      # db rows per psum tile
SU = 4096                        # db rows per super-unit (2 psum tiles)
N_SU = NDB // SU                 # 16 super-units
G = 8                            # fold degree: rows per z8 slot
SLOTW = SU // G                  # 512 z8 slots per su
FOLD_W = NDB // G                # 8192 z8 slots per q-tile
NCAND = 8 * G                    # 64 candidates = 8 needles x 8 rows/slot
PV = G * 2 * D + G               # 520: 8x [x(32) aux(32)] then 8x |x|^2

# static drain-path schedule per super-unit (16 entries).
# 3 = ACT evac + DVE bf16 fold chain (slot u covers rows su*4096 + u + 512m);
# 5 = DVE tensor_reduce(k=8) straight from PSUM (slot u covers 8 consecutive
#     rows: tile*2048 + 8*(u%256) + j).  Walrus bans TensorTensor with two
#     PSUM operands and ALL gpsimd tensor ops, so those are the only drains.
PATHS = [3] * 16


def build_nc(nq_core=NQ_CORE, ndb=NDB):
    n_qt = nq_core // 128
    n_su = ndb // SU

    nc = bass.Bass()
    qw = nc.declare_dram_parameter("qw_pack", [128, nq_core], BF16,
                                   isOutput=False)
    qf = nc.declare_dram_parameter("qf", [nq_core, D], F32, isOutput=False)
    qsq = nc.declare_dram_parameter("qsq", [nq_core, 1], F32, isOutput=False)
    dbA = nc.declare_dram_parameter("dbA", [DAUG, ndb // 2], BF16,
                                    isOutput=False)
    dbB = nc.declare_dram_parameter("dbB", [DAUG, ndb // 2], BF16,
                                    isOutput=False)
    pairt = nc.declare_dram_parameter("pair_table", [ndb // G, PV], F32,
                                      isOutput=False)
    out = nc.declare_dram_parameter("out", [nq_core, D], F32, isOutput=True)

    with TileContext(nc) as tc:
        with (
            tc.tile_pool(name="db", bufs=4) as dbp,
            tc.tile_pool(name="ps", bufs=2, space="PSUM") as psp,
            tc.tile_pool(name="za", bufs=2) as zap,      # ACT evac [128,4096]
            tc.tile_pool(name="z1", bufs=2) as z1p,      # L1 out [128,2048]
            tc.tile_pool(name="z2", bufs=2) as z2p,      # L2 out [128,1024]
            tc.tile_pool(name="zf", bufs=2) as zfp,      # z8 [128,8192] f32
            tc.tile_pool(name="sm", bufs=1) as sp,       # small tiles
            tc.tile_pool(name="g", bufs=1) as gp,        # gathered pairs
            tc.tile_pool(name="pr", bufs=1) as prp,      # refinement products
        ):
            # packed query weights, resident for the whole kernel
            qwt = sp.tile([128, nq_core], BF16, tag="qwt")
            nc.sync.dma_start(out=qwt[:], in_=qw[:])

            for t in range(n_qt):
                # per-qtile query-side tiles
                qft = sp.tile([128, D], F32, tag=f"qft{t % 2}")
                nc.sync.dma_start(out=qft[:], in_=qf[t * 128:(t + 1) * 128, :])
                qs = sp.tile([128, 1], F32, tag=f"qs{t % 2}")
                nc.sync.dma_start(out=qs[:], in_=qsq[t * 128:(t + 1) * 128, :])
                z8 = zfp.tile([128, FOLD_W], BF16, name=f"z8_{t % 2}",
                              tag=f"z8{t % 2}")

                lhsA = qwt[0:DAUG, t * 128:(t + 1) * 128]
                lhsB = qwt[64:64 + DAUG, t * 128:(t + 1) * 128]

                for su in range(n_su):
                    path = PATHS[su % len(PATHS)]
                    pss = []
                    for half in range(2):            # 2 psum tiles per su
                        tg = 2 * su + half           # global scan tile
                        st = dbp.tile([128, 1024], BF16)
                        nc.sync.dma_start(
                            out=st[0:DAUG, :],
                            in_=dbA[:, tg * 1024:(tg + 1) * 1024])
                        nc.sync.dma_start(
                            out=st[64:64 + DAUG, :],
                            in_=dbB[:, tg * 1024:(tg + 1) * 1024])
                        ps = psp.tile([128, TILE], F32)
                        for m in range(2):
                            sl = slice(m * 512, (m + 1) * 512)
                            nc.tensor.matmul(
                                out=ps[:, m * 512:(m + 1) * 512],
                                lhsT=lhsA, rhs=st[0:DAUG, sl],
                                start=True, stop=True, tile_position=(0, 0))
                            nc.tensor.matmul(
                                out=ps[:, 1024 + m * 512:1024 + (m + 1) * 512],
                                lhsT=lhsB, rhs=st[64:64 + DAUG, sl],
                                start=True, stop=True, tile_position=(64, 0))
                        pss.append(ps)

                    if path == 5:
                        # one grouped tensor_reduce per psum tile, k=8
                        for half in range(2):
                            nc.vector.tensor_reduce(
                                out=z8[:, (2 * su + half) * 256:
                                       (2 * su + half + 1) * 256],
                                in_=pss[half][:].rearrange(
                                    "p (u k) -> p u k", k=G),
                                axis=mybir.AxisListType.X,
                                op=mybir.AluOpType.max)
                    else:
                        # ACT evacuates both tiles with bf16 downcast
                        za = zap.tile([128, SU], BF16)
                        for half in range(2):
                            nc.scalar.copy(
                                out=za[:, half * TILE:(half + 1) * TILE],
                                in_=pss[half][:])
                        z1 = z1p.tile([128, TILE], BF16)
                        nc.vector.tensor_tensor(
                            out=z1[:], in0=za[:, 0:TILE], in1=za[:, TILE:SU],
                            op=mybir.AluOpType.max)
                        z2 = z2p.tile([128, TILE // 2], BF16)
                        nc.vector.tensor_tensor(
                            out=z2[:], in0=z1[:, 0:1024], in1=z1[:, 1024:2048],
                            op=mybir.AluOpType.max)
                        nc.vector.tensor_tensor(
                            out=z8[:, su * SLOTW:(su + 1) * SLOTW],
                            in0=z2[:, 0:512], in1=z2[:, 512:1024],
                            op=mybir.AluOpType.max)

                # ---- selection: top-8 z8 slots ----
                w8 = sp.tile([128, 8], BF16, tag=f"w8{t % 2}")
                nc.vector.max(out=w8[:], in_=z8[:])
                pos = sp.tile([128, 8], U32, tag=f"pos{t % 2}")
                nc.vector.max_index(out=pos[:], in_max=w8[:], in_values=z8[:])
                ji = sp.tile([128, 8], I32, tag=f"ji{t % 2}")
                nc.vector.tensor_copy(ji[:], pos[:])

                gxa = gp.tile([128, 8, PV], F32, tag="gxa")
                for i in range(8):
                    nc.gpsimd.indirect_dma_start(
                        out=gxa[:, i, :], out_offset=None, in_=pairt[:],
                        in_offset=bass.IndirectOffsetOnAxis(
                            ap=ji[:, i:i + 1], axis=0))

                # ---- exact f32 refinement over the 64 candidates ----
                base = gxa[:, :, 0:2 * G * D].rearrange(
                    "p c (h v) -> p c h v", h=G)
                gx = base[:, :, :, 0:D]
                ga = base[:, :, :, D:2 * D]
                xsq = gxa[:, :, 2 * G * D:2 * G * D + G]    # [128, 8, G]
                pr = prp.tile([128, 8, G, D], F32, tag="pr")
                nc.vector.tensor_tensor(
                    out=pr[:], in0=gx,
                    in1=qft[:].unsqueeze(1).unsqueeze(1)
                              .to_broadcast([128, 8, G, D]),
                    op=mybir.AluOpType.mult)
                dots = sp.tile([128, 8, G], F32, tag=f"dots{t % 2}")
                nc.vector.tensor_reduce(out=dots[:], in_=pr[:],
                                        axis=mybir.AxisListType.X,
                                        op=mybir.AluOpType.add)
                # neg2 = 2*dots - xsq  (dsq = qsq - neg2)
                neg2 = sp.tile([128, NCAND], F32, tag=f"neg2{t % 2}")
                nc.vector.scalar_tensor_tensor(
                    out=neg2[:].rearrange("p (c h) -> p c h", h=G),
                    in0=dots[:], scalar=2.0, in1=xsq,
                    op0=mybir.AluOpType.mult,
                    op1=mybir.AluOpType.subtract)
                t8 = sp.tile([128, 8], F32, tag=f"t8{t % 2}")
                nc.vector.max(out=t8[:], in_=neg2[:])
                mask = sp.tile([128, NCAND], F32, tag=f"mask{t % 2}")
                nc.vector.tensor_scalar(mask[:], neg2[:], t8[:, 4:5], None,
                                        op0=mybir.AluOpType.is_ge)
                dsq = sp.tile([128, NCAND], F32, tag=f"dsq{t % 2}")
                nc.vector.tensor_scalar(dsq[:], neg2[:], -1.0, qs[:, 0:1],
                                        op0=mybir.AluOpType.mult,
                                        op1=mybir.AluOpType.add)
                nc.vector.tensor_scalar_max(dsq[:], dsq[:], 0.0)
                dist = sp.tile([128, NCAND], F32, tag=f"dist{t % 2}")
                nc.scalar.sqrt(out=dist[:], in_=dsq[:])
                nc.vector.tensor_scalar_add(dist[:], dist[:], EPS)
                rec = sp.tile([128, NCAND], F32, tag=f"rec{t % 2}")
                nc.vector.reciprocal(out=rec[:], in_=dist[:])
                wgt = sp.tile([128, NCAND], F32, tag=f"wgt{t % 2}")
                nc.vector.tensor_tensor(out=wgt[:], in0=rec[:], in1=mask[:],
                                        op=mybir.AluOpType.mult)
                wsum = sp.tile([128, 1], F32, tag=f"wsum{t % 2}")
                nc.vector.tensor_reduce(out=wsum[:], in_=wgt[:],
                                        axis=mybir.AxisListType.X,
                                        op=mybir.AluOpType.add)
                winv = sp.tile([128, 1], F32, tag=f"winv{t % 2}")
                nc.vector.reciprocal(out=winv[:], in_=wsum[:])

                # weighted sum of gathered aux rows (on Pool to unload DVE)
                prod = prp.tile([128, 8, G, D], F32, tag="prod")
                nc.vector.tensor_tensor(
                    out=prod[:], in0=ga,
                    in1=wgt[:].rearrange("p (c h) -> p c h", h=G)
                              .unsqueeze(-1).to_broadcast([128, 8, G, D]),
                    op=mybir.AluOpType.mult)
                acc = sp.tile([128, D], F32, tag=f"accr{t % 2}")
                nc.vector.tensor_reduce(
                    out=acc[:],
                    in_=prod[:].rearrange("p i h a -> p a (i h)"),
                    axis=mybir.AxisListType.X, op=mybir.AluOpType.add)
                outt = sp.tile([128, D], F32, tag=f"outt{t % 2}")
                nc.vector.tensor_scalar(outt[:], acc[:], winv[:, 0:1], None,
                                        op0=mybir.AluOpType.mult)
                nc.sync.dma_start(out=out[t * 128:(t + 1) * 128, :],
                                  in_=outt[:])

    split_multi_waits(nc)
    return nc


def split_multi_waits(nc):
    """The walrus build in this container supports a single sync-wait per
    instruction; Tile's tail drain carries one wait per live proc.  Split
    any multi-wait instruction into single-wait NoOps ahead of it."""
    for f in nc.m.functions:
        for blk in f.blocks:
            newinsts = []
            for ins in blk.instructions:
                si = ins.sync_info
                if si is not None and si.on_wait and len(si.on_wait) > 1:
                    waits = list(si.on_wait)
                    for k, w in enumerate(waits[:-1]):
                        nop = mybir.InstNoOp(name=f"{ins.name}-ws{k}", ins=[],
                                             outs=[])
                        nop.engine = ins.engine
                        nop.sync_info = mybir.SyncInfo(on_wait=[w], on_update=[])
                        newinsts.append(nop)
                    ins.sync_info = mybir.SyncInfo(on_wait=[waits[-1]],
                                                   on_update=list(si.on_update))
                newinsts.append(ins)
            blk.instructions = newinsts


def make_in_maps(embedding_features, db_embedding, auxiliary_features):
    q = np.ascontiguousarray(np.asarray(embedding_features, dtype=np.float32))
    db = np.ascontiguousarray(np.asarray(db_embedding, dtype=np.float32))
    aux = np.ascontiguousarray(np.asarray(auxiliary_features, dtype=np.float32))
    ndb = db.shape[0]
    nq_core = q.shape[0] // N_CORES
    bf = ml_dtypes.bfloat16
    bias = -0.5 * (db * db).sum(1)                      # exact f32
    b_hi = bias.astype(bf).astype(np.float32)
    b_lo = (bias - b_hi).astype(bf)
    dbT_aug = np.ascontiguousarray(np.concatenate(
        [db.T.astype(bf), b_hi.astype(bf)[None, :], b_lo[None, :]], axis=0,
        dtype=bf))                                      # [34, ndb]
    # split into the two row-group slabs: scan tile tg covers db rows
    # [tg*2048, (tg+1)*2048); dbA carries its first 1024, dbB the second.
    cols = np.arange(ndb)
    tg_i = cols // TILE
    off = cols % TILE
    a_cols = (tg_i * 1024 + off)[off < 1024]
    b_cols = (tg_i * 1024 + (off - 1024))[off >= 1024]
    dbA = np.zeros((DAUG, ndb // 2), bf)
    dbB = np.zeros((DAUG, ndb // 2), bf)
    dbA[:, a_cols] = dbT_aug[:, off < 1024]
    dbB[:, b_cols] = dbT_aug[:, off >= 1024]
    dbA = np.ascontiguousarray(dbA)
    dbB = np.ascontiguousarray(dbB)

    # pair table: slot map depends on the drain path of the slot's su.
    # path 3: slot s = su*SLOTW + u covers db rows su*SU + u + 512*m
    # path 5: slot s covers 8 consecutive rows tile*2048 + 8*(u % 256) + m
    idx = np.arange(ndb // G)
    su_i = idx // SLOTW
    u_i = idx % SLOTW
    path_i = np.array([PATHS[s % len(PATHS)] for s in range(ndb // SU)])[su_i]
    dbsq = (db * db).sum(1)
    pair_table = np.zeros((ndb // G, PV), np.float32)
    for m in range(G):
        jm3 = su_i * SU + u_i + 512 * m
        tile_i = 2 * su_i + u_i // 256
        jm5 = tile_i * TILE + 8 * (u_i % 256) + m
        jm = np.where(path_i == 5, jm5, jm3)
        pair_table[:, 2 * m * D:(2 * m + 1) * D] = db[jm]
        pair_table[:, (2 * m + 1) * D:(2 * m + 2) * D] = aux[jm]
        pair_table[:, 2 * G * D + m] = dbsq[jm]
    pair_table = np.ascontiguousarray(pair_table)

    in_maps = []
    for c in range(N_CORES):
        qs = q[c * nq_core:(c + 1) * nq_core]
        qT_aug = np.concatenate(
            [qs.T.astype(bf), np.ones((2, nq_core), bf)], axis=0, dtype=bf)
        qw_pack = np.zeros((128, nq_core), bf)
        qw_pack[0:DAUG] = qT_aug
        qw_pack[64:64 + DAUG] = qT_aug
        qw_pack = np.ascontiguousarray(qw_pack)
        qsq = np.ascontiguousarray((qs * qs).sum(1).reshape(nq_core, 1)
                                   ).astype(np.float32)
        in_maps.append({"qw_pack": qw_pack, "qf": qs, "qsq": qsq,
                        "dbA": dbA, "dbB": dbB, "pair_table": pair_table})
    return in_maps


_NC_CACHE = {}


def get_nc(nq_core=NQ_CORE, ndb=NDB):
    key = (nq_core, ndb)
    if key not in _NC_CACHE:
        _NC_CACHE[key] = build_nc(nq_core, ndb)
    return _NC_CACHE[key]


def kernel(embedding_features, db_embedding, auxiliary_features):
    in_maps = make_in_maps(embedding_features, db_embedding, auxiliary_features)
    nc = get_nc()
    res = run_bass_kernel_spmd(nc, in_maps, list(range(N_CORES)))
    return np.concatenate([res.results[c]["out"] for c in range(N_CORES)],
                          axis=0).astype(np.float32)


# revision 10
# speedup vs baseline: 1.1370x; 1.1370x over previous
"""Trainium2 Bass kernel for nn_BaseEmbedder (retrieval_knn).

For each of 4096 query embeddings: find the 5 nearest of 65536 db embeddings
(Euclidean) and produce the inverse-distance-weighted sum of their auxiliary
features.  SPMD on 8 NeuronCores: queries sharded 512/core, db+aux replicated.

v2 design (vs the single-row-group v1):
  - Scan (bf16): negS[q,j] = q.x_j - 0.5|x_j|^2 via K=34 augmented bf16
    matmuls.  The PE runs TWO matmuls concurrently via 2-way row-group
    packing: weights live at array rows 0-33 (tile_position (0,0)) and rows
    64-97 (tile_position (64,0)).  db columns are split host-side into two
    DRAM slabs (dbA = first 1024 of each 2048-row tile, dbB = second 1024)
    and streamed into one [128, 1024] SBUF tile per scan tile (partitions
    0:34 and 64:98).
  - PSUM drain runs on three parallel paths, chosen statically per 4096-col
    "super unit" (su = 2 psum tiles):
      P2: DVE folds directly from PSUM (TT max of tile halves), Pool (gpsimd)
          finishes the bf16 fold chain.
      P3: ACT evacuates both tiles to bf16, DVE runs the whole fold chain.
      P4: ACT evacuates, DVE does the first bf16 fold, Pool finishes.
    All paths produce the same slot map: z8 slot (su, u) covers db rows
    su*4096 + u + 512*m, m=0..7 (every fold level halves the linear psum
    order), so the host pair table is path-independent.
  - Selection: MAX8 + FIND_INDEX8 over z8 [128, 8192] f32 per q-tile, as in
    v1 (custom ops are f32-only on HW).
  - Exact refinement (f32): per winning slot a host-prepared row
    [8x (x,aux), 8x |x|^2] is gathered via per-partition indirect DMA; exact
    distances for all 64 candidates are recomputed; top-5 by threshold;
    weights 1/(d+eps) normalized; weighted aux sum.  The aux-side product and
    reduction run on Pool (gpsimd) to unload DVE.

The bf16 scan only nominates candidates; all selection/weight math is exact
f32, so the result matches the f32 reference to ~1e-6.
"""

import numpy as np
import ml_dtypes

from concourse import bass, mybir
from concourse.tile import TileContext
from concourse.bass_utils import run_bass_kernel_spmd

F32 = mybir.dt.float32
BF16 = mybir.dt.bfloat16
U32 = mybir.dt.uint32
I32 = mybir.dt.int32

N_CORES = 8
NQ = 4096
NDB = 65536
D = 32
DAUG = 34   # 32 dims + bias row + bias-residual row (bf16 split)
K = 5
EPS = 1e-6

NQ_CORE = NQ // N_CORES          # 512
N_QT = NQ_CORE // 128            # 4 q-tiles
TILE = 2048                      # db rows per psum tile
SU = 4096                        # db rows per super-unit (2 psum tiles)
N_SU = NDB // SU                 # 16 super-units
G = 8                            # fold degree: rows per z8 slot
SLOTW = SU // G                  # 512 z8 slots per su
FOLD_W = NDB // G                # 8192 z8 slots per q-tile
NCAND = 8 * G                    # 64 candidates = 8 needles x 8 rows/slot
PV = G * 2 * D + G               # 520: 8x [x(32) aux(32)] then 8x |x|^2

# static drain-path schedule per super-unit (16 entries).
# 3 = ACT evac + DVE bf16 fold chain (slot u covers rows su*4096 + u + 512m);
# 5 = DVE tensor_reduce(k=8) straight from PSUM (slot u covers 8 consecutive
#     rows: tile*2048 + 8*(u%256) + j).  Walrus bans TensorTensor with two
#     PSUM operands and ALL gpsimd tensor ops, so those are the only drains.
PATHS = [3] * 16


def build_nc(nq_core=NQ_CORE, ndb=NDB):
    n_qt = nq_core // 128
    n_su = ndb // SU

    nc = bass.Bass()
    qw = nc.declare_dram_parameter("qw_pack", [128, nq_core], BF16,
                                   isOutput=False)
    qf = nc.declare_dram_parameter("qf", [nq_core, D], F32, isOutput=False)
    qsq = nc.declare_dram_parameter("qsq", [nq_core, 1], F32, isOutput=False)
    dbA = nc.declare_dram_parameter("dbA", [DAUG, ndb // 2], BF16,
                                    isOutput=False)
    dbB = nc.declare_dram_parameter("dbB", [DAUG, ndb // 2], BF16,
                                    isOutput=False)
    pairt = nc.declare_dram_parameter("pair_table", [ndb // G, PV], F32,
                                      isOutput=False)
    out = nc.declare_dram_parameter("out", [nq_core, D], F32, isOutput=True)

    with TileContext(nc) as tc:
        with (
            tc.tile_pool(name="db", bufs=4) as dbp,
            tc.tile_pool(name="ps", bufs=2, space="PSUM") as psp,
            tc.tile_pool(name="za", bufs=2) as zap,      # ACT evac [128,4096]
            tc.tile_pool(name="z1", bufs=2) as z1p,      # L1 out [128,2048]
            tc.tile_pool(name="z2", bufs=2) as z2p,      # L2 out [128,1024]
            tc.tile_pool(name="zf", bufs=2) as zfp,      # z8 [128,8192] f32
            tc.tile_pool(name="sm", bufs=1) as sp,       # small tiles
            tc.tile_pool(name="g", bufs=1) as gp,        # gathered pairs
            tc.tile_pool(name="pr", bufs=1) as prp,      # refinement products
        ):
            # packed query weights, resident for the whole kernel
            qwt = sp.tile([128, nq_core], BF16, tag="qwt")
            nc.sync.dma_start(out=qwt[:], in_=qw[:])

            for t in range(n_qt):
                # per-qtile query-side tiles
                qft = sp.tile([128, D], F32, tag=f"qft{t % 2}")
                nc.sync.dma_start(out=qft[:], in_=qf[t * 128:(t + 1) * 128, :])
                qs = sp.tile([128, 1], F32, tag=f"qs{t % 2}")
                nc.sync.dma_start(out=qs[:], in_=qsq[t * 128:(t + 1) * 128, :])
                z8 = zfp.tile([128, FOLD_W], BF16, name=f"z8_{t % 2}",
                              tag=f"z8{t % 2}")

                lhsA = qwt[0:DAUG, t * 128:(t + 1) * 128]
                lhsB = qwt[64:64 + DAUG, t * 128:(t + 1) * 128]

                for su in range(n_su):
                    path = PATHS[su % len(PATHS)]
                    # one stream tile per su (2 scan tiles); B-slab issued
                    # first on the gpsimd queue, A-slab on sync, so neither
                    # DMA queue saturates and the B matmuls aren't starved.
                    st = dbp.tile([128, TILE], BF16)
                    nc.gpsimd.dma_start(
                        out=st[64:64 + DAUG, :],
                        in_=dbB[:, su * TILE:(su + 1) * TILE])
                    nc.sync.dma_start(
                        out=st[0:DAUG, :],
                        in_=dbA[:, su * TILE:(su + 1) * TILE])
                    pss = []
                    for half in range(2):            # 2 psum tiles per su
                        ps = psp.tile([128, TILE], F32)
                        for m in range(2):
                            sl = slice(half * 1024 + m * 512,
                                       half * 1024 + (m + 1) * 512)
                            nc.tensor.matmul(
                                out=ps[:, 1024 + m * 512:1024 + (m + 1) * 512],
                                lhsT=lhsB, rhs=st[64:64 + DAUG, sl],
                                start=True, stop=True, tile_position=(64, 0))
                        for m in range(2):
                            sl = slice(half * 1024 + m * 512,
                                       half * 1024 + (m + 1) * 512)
                            nc.tensor.matmul(
                                out=ps[:, m * 512:(m + 1) * 512],
                                lhsT=lhsA, rhs=st[0:DAUG, sl],
                                start=True, stop=True, tile_position=(0, 0))
                        pss.append(ps)

                    if path == 5:
                        # one grouped tensor_reduce per psum tile, k=8
                        for half in range(2):
                            nc.vector.tensor_reduce(
                                out=z8[:, (2 * su + half) * 256:
                                       (2 * su + half + 1) * 256],
                                in_=pss[half][:].rearrange(
                                    "p (u k) -> p u k", k=G),
                                axis=mybir.AxisListType.X,
                                op=mybir.AluOpType.max)
                    else:
                        # ACT evacuates both tiles with bf16 downcast
                        za = zap.tile([128, SU], BF16)
                        for half in range(2):
                            nc.scalar.copy(
                                out=za[:, half * TILE:(half + 1) * TILE],
                                in_=pss[half][:])
                        z1 = z1p.tile([128, TILE], BF16)
                        nc.vector.tensor_tensor(
                            out=z1[:], in0=za[:, 0:TILE], in1=za[:, TILE:SU],
                            op=mybir.AluOpType.max)
                        z2 = z2p.tile([128, TILE // 2], BF16)
                        nc.vector.tensor_tensor(
                            out=z2[:], in0=z1[:, 0:1024], in1=z1[:, 1024:2048],
                            op=mybir.AluOpType.max)
                        nc.vector.tensor_tensor(
                            out=z8[:, su * SLOTW:(su + 1) * SLOTW],
                            in0=z2[:, 0:512], in1=z2[:, 512:1024],
                            op=mybir.AluOpType.max)

                # ---- selection: top-8 z8 slots ----
                w8 = sp.tile([128, 8], BF16, tag=f"w8{t % 2}")
                nc.vector.max(out=w8[:], in_=z8[:])
                pos = sp.tile([128, 8], U32, tag=f"pos{t % 2}")
                nc.vector.max_index(out=pos[:], in_max=w8[:], in_values=z8[:])
                ji = sp.tile([128, 8], I32, tag=f"ji{t % 2}")
                nc.vector.tensor_copy(ji[:], pos[:])

                gxa = gp.tile([128, 8, PV], F32, tag="gxa")
                for i in range(8):
                    nc.gpsimd.indirect_dma_start(
                        out=gxa[:, i, :], out_offset=None, in_=pairt[:],
                        in_offset=bass.IndirectOffsetOnAxis(
                            ap=ji[:, i:i + 1], axis=0))

                # ---- exact f32 refinement over the 64 candidates ----
                base = gxa[:, :, 0:2 * G * D].rearrange(
                    "p c (h v) -> p c h v", h=G)
                gx = base[:, :, :, 0:D]
                ga = base[:, :, :, D:2 * D]
                xsq = gxa[:, :, 2 * G * D:2 * G * D + G]    # [128, 8, G]
                pr = prp.tile([128, 8, G, D], F32, tag="pr")
                nc.vector.tensor_tensor(
                    out=pr[:], in0=gx,
                    in1=qft[:].unsqueeze(1).unsqueeze(1)
                              .to_broadcast([128, 8, G, D]),
                    op=mybir.AluOpType.mult)
                dots = sp.tile([128, 8, G], F32, tag=f"dots{t % 2}")
                nc.vector.tensor_reduce(out=dots[:], in_=pr[:],
                                        axis=mybir.AxisListType.X,
                                        op=mybir.AluOpType.add)
                # neg2 = 2*dots - xsq  (dsq = qsq - neg2)
                neg2 = sp.tile([128, NCAND], F32, tag=f"neg2{t % 2}")
                nc.vector.scalar_tensor_tensor(
                    out=neg2[:].rearrange("p (c h) -> p c h", h=G),
                    in0=dots[:], scalar=2.0, in1=xsq,
                    op0=mybir.AluOpType.mult,
                    op1=mybir.AluOpType.subtract)
                t8 = sp.tile([128, 8], F32, tag=f"t8{t % 2}")
                nc.vector.max(out=t8[:], in_=neg2[:])
                mask = sp.tile([128, NCAND], F32, tag=f"mask{t % 2}")
                nc.vector.tensor_scalar(mask[:], neg2[:], t8[:, 4:5], None,
                                        op0=mybir.AluOpType.is_ge)
                dsq = sp.tile([128, NCAND], F32, tag=f"dsq{t % 2}")
                nc.vector.tensor_scalar(dsq[:], neg2[:], -1.0, qs[:, 0:1],
                                        op0=mybir.AluOpType.mult,
                                        op1=mybir.AluOpType.add)
                nc.vector.tensor_scalar_max(dsq[:], dsq[:], 0.0)
                dist = sp.tile([128, NCAND], F32, tag=f"dist{t % 2}")
                nc.scalar.sqrt(out=dist[:], in_=dsq[:])
                nc.vector.tensor_scalar_add(dist[:], dist[:], EPS)
                rec = sp.tile([128, NCAND], F32, tag=f"rec{t % 2}")
                nc.vector.reciprocal(out=rec[:], in_=dist[:])
                wgt = sp.tile([128, NCAND], F32, tag=f"wgt{t % 2}")
                nc.vector.tensor_tensor(out=wgt[:], in0=rec[:], in1=mask[:],
                                        op=mybir.AluOpType.mult)
                wsum = sp.tile([128, 1], F32, tag=f"wsum{t % 2}")
                nc.vector.tensor_reduce(out=wsum[:], in_=wgt[:],
                                        axis=mybir.AxisListType.X,
                                        op=mybir.AluOpType.add)
                winv = sp.tile([128, 1], F32, tag=f"winv{t % 2}")
                nc.vector.reciprocal(out=winv[:], in_=wsum[:])

                # weighted sum of gathered aux rows (on Pool to unload DVE)
                prod = prp.tile([128, 8, G, D], F32, tag="prod")
                nc.vector.tensor_tensor(
                    out=prod[:], in0=ga,
                    in1=wgt[:].rearrange("p (c h) -> p c h", h=G)
                              .unsqueeze(-1).to_broadcast([128, 8, G, D]),
                    op=mybir.AluOpType.mult)
                acc = sp.tile([128, D], F32, tag=f"accr{t % 2}")
                nc.vector.tensor_reduce(
                    out=acc[:],
                    in_=prod[:].rearrange("p i h a -> p a (i h)"),
                    axis=mybir.AxisListType.X, op=mybir.AluOpType.add)
                outt = sp.tile([128, D], F32, tag=f"outt{t % 2}")
                nc.vector.tensor_scalar(outt[:], acc[:], winv[:, 0:1], None,
                                        op0=mybir.AluOpType.mult)
                nc.sync.dma_start(out=out[t * 128:(t + 1) * 128, :],
                                  in_=outt[:])

    split_multi_waits(nc)
    return nc


def split_multi_waits(nc):
    """The walrus build in this container supports a single sync-wait per
    instruction; Tile's tail drain carries one wait per live proc.  Split
    any multi-wait instruction into single-wait NoOps ahead of it."""
    for f in nc.m.functions:
        for blk in f.blocks:
            newinsts = []
            for ins in blk.instructions:
                si = ins.sync_info
                if si is not None and si.on_wait and len(si.on_wait) > 1:
                    waits = list(si.on_wait)
                    for k, w in enumerate(waits[:-1]):
                        nop = mybir.InstNoOp(name=f"{ins.name}-ws{k}", ins=[],
                                             outs=[])
                        nop.engine = ins.engine
                        nop.sync_info = mybir.SyncInfo(on_wait=[w], on_update=[])
                        newinsts.append(nop)
                    ins.sync_info = mybir.SyncInfo(on_wait=[waits[-1]],
                                                   on_update=list(si.on_update))
                newinsts.append(ins)
            blk.instructions = newinsts


def make_in_maps(embedding_features, db_embedding, auxiliary_features):
    q = np.ascontiguousarray(np.asarray(embedding_features, dtype=np.float32))
    db = np.ascontiguousarray(np.asarray(db_embedding, dtype=np.float32))
    aux = np.ascontiguousarray(np.asarray(auxiliary_features, dtype=np.float32))
    ndb = db.shape[0]
    nq_core = q.shape[0] // N_CORES
    bf = ml_dtypes.bfloat16
    bias = -0.5 * (db * db).sum(1)                      # exact f32
    b_hi = bias.astype(bf).astype(np.float32)
    b_lo = (bias - b_hi).astype(bf)
    dbT_aug = np.ascontiguousarray(np.concatenate(
        [db.T.astype(bf), b_hi.astype(bf)[None, :], b_lo[None, :]], axis=0,
        dtype=bf))                                      # [34, ndb]
    # split into the two row-group slabs: scan tile tg covers db rows
    # [tg*2048, (tg+1)*2048); dbA carries its first 1024, dbB the second.
    cols = np.arange(ndb)
    tg_i = cols // TILE
    off = cols % TILE
    a_cols = (tg_i * 1024 + off)[off < 1024]
    b_cols = (tg_i * 1024 + (off - 1024))[off >= 1024]
    dbA = np.zeros((DAUG, ndb // 2), bf)
    dbB = np.zeros((DAUG, ndb // 2), bf)
    dbA[:, a_cols] = dbT_aug[:, off < 1024]
    dbB[:, b_cols] = dbT_aug[:, off >= 1024]
    dbA = np.ascontiguousarray(dbA)
    dbB = np.ascontiguousarray(dbB)

    # pair table: slot map depends on the drain path of the slot's su.
    # path 3: slot s = su*SLOTW + u covers db rows su*SU + u + 512*m
    # path 5: slot s covers 8 consecutive rows tile*2048 + 8*(u % 256) + m
    idx = np.arange(ndb // G)
    su_i = idx // SLOTW
    u_i = idx % SLOTW
    path_i = np.array([PATHS[s % len(PATHS)] for s in range(ndb // SU)])[su_i]
    dbsq = (db * db).sum(1)
    pair_table = np.zeros((ndb // G, PV), np.float32)
    for m in range(G):
        jm3 = su_i * SU + u_i + 512 * m
        tile_i = 2 * su_i + u_i // 256
        jm5 = tile_i * TILE + 8 * (u_i % 256) + m
        jm = np.where(path_i == 5, jm5, jm3)
        pair_table[:, 2 * m * D:(2 * m + 1) * D] = db[jm]
        pair_table[:, (2 * m + 1) * D:(2 * m + 2) * D] = aux[jm]
        pair_table[:, 2 * G * D + m] = dbsq[jm]
    pair_table = np.ascontiguousarray(pair_table)

    in_maps = []
    for c in range(N_CORES):
        qs = q[c * nq_core:(c + 1) * nq_core]
        qT_aug = np.concatenate(
            [qs.T.astype(bf), np.ones((2, nq_core), bf)], axis=0, dtype=bf)
        qw_pack = np.zeros((128, nq_core), bf)
        qw_pack[0:DAUG] = qT_aug
        qw_pack[64:64 + DAUG] = qT_aug
        qw_pack = np.ascontiguousarray(qw_pack)
        qsq = np.ascontiguousarray((qs * qs).sum(1).reshape(nq_core, 1)
                                   ).astype(np.float32)
        in_maps.append({"qw_pack": qw_pack, "qf": qs, "qsq": qsq,
                        "dbA": dbA, "dbB": dbB, "pair_table": pair_table})
    return in_maps


_NC_CACHE = {}


def get_nc(nq_core=NQ_CORE, ndb=NDB):
    key = (nq_core, ndb)
    if key not in _NC_CACHE:
        _NC_CACHE[key] = build_nc(nq_core, ndb)
    return _NC_CACHE[key]


def kernel(embedding_features, db_embedding, auxiliary_features):
    in_maps = make_in_maps(embedding_features, db_embedding, auxiliary_features)
    nc = get_nc()
    res = run_bass_kernel_spmd(nc, in_maps, list(range(N_CORES)))
    return np.concatenate([res.results[c]["out"] for c in range(N_CORES)],
                          axis=0).astype(np.float32)
